# revision 1
# baseline (speedup 1.0000x reference)
"""Causal MHA (B=4, L=2048, D=1024, H=16) on 8 NeuronCores.

Sharding: core c -> (batch b = c//2, head-group g = c%2). Data-parallel over
the 4 batches, tensor-parallel over heads (8 heads per core): wq/wk/wv
column-parallel, wo row-parallel. Each core returns a partial [L, D] output;
the host sums the two head-group partials per batch and adds wo_b.

Per-core device kernel (all matmuls fp32r: 1 cyc/row at N>=256, ~1.5e-4 rel):
  A) QT = (wq_g*0.125) @ q_b.T + bq  -> [512, 2048] SBUF (head dims on parts)
     KT likewise (unscaled).  V_aug = q_b @ wv_aug.T + vb -> [2048, 520] DRAM
     (per head: 64 dims + a ones column -> fused softmax denominator).
  B) per head h, per 512-wide q-slice: S.T[keys,q] = KT_h.T-slice @ QT_h
     (causal-trimmed N), exp on ScalarE, tri-mask on the diagonal 128-block,
     AV: psum[65, q] += V_aug_h[kb].T @ P.T  (row 64 = denominator).
     Normalize rows 0..63 by 1/denom (DVE recip + GpSimd partition_broadcast
     + DVE mul) -> ctxT [512, 2048] spilled to DRAM.
  C) out_partial[t, :] = sum_c ctxT[c, t-tile].T @ woT[c] -> [2048, 1024] f32.
"""

import numpy as np

import concourse.bacc as bacc
import concourse.bass as bass
import concourse.mybir as mybir
import concourse.tile as tile
from concourse.bass_utils import run_bass_kernel_spmd

F32 = mybir.dt.float32
F32R = mybir.dt.float32r

B, L, D, H, DK = 4, 2048, 1024, 16, 64
HD = 8            # heads per core
GW = 512          # head-group width (8 heads * 64)
AUGW = HD * (DK + 1)  # 520: per head 64 dims + ones col (ones LAST per head)
NCH = D // 128    # 8 contraction chunks
QS = 512          # q-slice width in attention
NQS = L // QS     # 4
NKB = L // 128    # 16 key blocks
NTT = L // 128    # 16 token tiles


def _r(ap):
    return ap


def _build_nc(dbg=False, phases="ABC"):
    nc = bacc.Bacc("TRN2", target_bir_lowering=False, debug=False, num_devices=8)

    xq = nc.dram_tensor("xq", [D, L], F32R, kind="ExternalInput").ap()
    xk = nc.dram_tensor("xk", [D, L], F32R, kind="ExternalInput").ap()
    xv = nc.dram_tensor("xv", [D, L], F32R, kind="ExternalInput").ap()
    wq = nc.dram_tensor("wq", [D, GW], F32R, kind="ExternalInput").ap()
    wk = nc.dram_tensor("wk", [D, GW], F32R, kind="ExternalInput").ap()
    wv = nc.dram_tensor("wv", [D, AUGW], F32R, kind="ExternalInput").ap()
    wo = nc.dram_tensor("wo", [GW, D], F32R, kind="ExternalInput").ap()
    bq = nc.dram_tensor("bq", [128, 4], F32, kind="ExternalInput").ap()
    bk = nc.dram_tensor("bk", [128, 4], F32, kind="ExternalInput").ap()
    vb = nc.dram_tensor("vb", [AUGW], F32, kind="ExternalInput").ap()
    msk = nc.dram_tensor("msk", [128, 128], F32, kind="ExternalInput").ap()
    outp = nc.dram_tensor("outp", [L, D], F32, kind="ExternalOutput").ap()
    if dbg:
        qt_dbg = nc.dram_tensor("qt_dbg", [128, 4 * L], F32, kind="ExternalOutput").ap()
        kt_dbg = nc.dram_tensor("kt_dbg", [128, 4 * L], F32, kind="ExternalOutput").ap()
        vg_dbg = nc.dram_tensor("vg_dbg", [L, AUGW], F32, kind="ExternalOutput").ap()
        ctx_dbg = nc.dram_tensor("ctx_dbg", [GW, L], F32, kind="ExternalOutput").ap()

    with tile.TileContext(nc) as tc:
        with (
            tc.tile_pool(name="persist", bufs=1) as persist,
            tc.tile_pool(name="xin", bufs=10) as xinp,
            tc.tile_pool(name="work", bufs=4) as workp,
            tc.tile_pool(name="pt", bufs=5) as ptp,
            tc.tile_pool(name="vh", bufs=2) as vhp,
            tc.tile_pool(name="small", bufs=4) as smallp,
            tc.tile_pool(name="outs", bufs=3) as outsp,
            tc.tile_pool(name="psA", bufs=6, space="PSUM") as psA,
            tc.tile_pool(name="psC", bufs=2, space="PSUM") as psC,
            tc.tile_pool(name="dram", bufs=1, space="DRAM") as dramp,
            tc.tile_pool(name="dnb", bufs=4, space="DRAM") as dnbp,
        ):
            # ---- persistent SBUF ----
            wq_s = persist.tile([128, NCH, GW], F32R, tag="wq")
            wk_s = persist.tile([128, NCH, GW], F32R, tag="wk")
            wv_s = persist.tile([128, NCH, AUGW], F32R, tag="wv")
            wo_s = persist.tile([128, 4, D], F32R, tag="wo")
            qt_s = persist.tile([128, 4, L], F32R, tag="qt")
            kt_s = persist.tile([128, 4, L], F32R, tag="kt")
            bq_s = persist.tile([128, 4], F32, tag="bq")
            bk_s = persist.tile([128, 4], F32, tag="bk")
            vb_s = persist.tile([128, AUGW], F32, tag="vb")
            msk_s = persist.tile([128, 128], F32, tag="msk")

            vg_d = dramp.tile([L, AUGW], F32R, tag="vg")
            ctx_d = dramp.tile([GW, L], F32R, tag="ctx")

            for c in range(NCH):
                nc.sync.dma_start(wq_s[:, c, :], wq[c * 128:(c + 1) * 128, :])
                nc.sync.dma_start(wk_s[:, c, :], wk[c * 128:(c + 1) * 128, :])
                nc.sync.dma_start(wv_s[:, c, :], wv[c * 128:(c + 1) * 128, :])
            for c in range(4):
                nc.sync.dma_start(wo_s[:, c, :], wo[c * 128:(c + 1) * 128, :])
            nc.sync.dma_start(bq_s[:, :], bq[:, :])
            nc.sync.dma_start(bk_s[:, :], bk[:, :])
            nc.sync.dma_start(msk_s[:, :], msk[:, :])
            vb_bcast = bass.AP(tensor=vb.tensor, offset=vb.offset,
                               ap=[[0, 128], [1, AUGW]])
            nc.gpsimd.dma_start(vb_s[:, :], vb_bcast)

            # ---- phase A: projections ----
            for n in range(4):  # 512-token slice
                for (src, w_s, dst, b_s) in ((xq, wq_s, qt_s, bq_s),
                                             (xk, wk_s, kt_s, bk_s)):
                    xt = []
                    for c in range(NCH):
                        t = xinp.tile([128, 512], F32R, tag="xin")
                        nc.sync.dma_start(
                            t[:, :], src[c * 128:(c + 1) * 128,
                                         n * 512:(n + 1) * 512])
                        xt.append(t)
                    pss = [psA.tile([128, 512], F32, tag="ps", name=f"psA{i}") for i in range(4)]
                    for c in range(NCH):
                        for m in range(4):
                            nc.tensor.matmul(
                                pss[m][:, :],
                                _r(w_s[:, c, m * 128:(m + 1) * 128]),
                                _r(xt[c][:, :]),
                                start=(c == 0), stop=(c == NCH - 1))
                    for m in range(4):
                        nc.vector.tensor_scalar_add(
                            dst[:, m, n * 512:(n + 1) * 512],
                            pss[m][:, :], b_s[:, m:m + 1])
                # V_aug
                xt = []
                for c in range(NCH):
                    t = xinp.tile([128, 512], F32R, tag="xin")
                    nc.sync.dma_start(
                        t[:, :], xv[c * 128:(c + 1) * 128,
                                    n * 512:(n + 1) * 512])
                    xt.append(t)
                for tt in range(4):  # token tile within slice
                    for hf in range(2):
                        ps = psA.tile([128, 260], F32, tag="ps")
                        for c in range(NCH):
                            nc.tensor.matmul(
                                ps[:, :],
                                _r(xt[c][:, tt * 128:(tt + 1) * 128]),
                                _r(wv_s[:, c, hf * 260:(hf + 1) * 260]),
                                start=(c == 0), stop=(c == NCH - 1))
                        vst = workp.tile([128, 260], F32R, tag="vst")
                        nc.vector.tensor_add(
                            vst[:, :], ps[:, :],
                            vb_s[:, hf * 260:(hf + 1) * 260])
                        nc.sync.dma_start(
                            vg_d[(n * 4 + tt) * 128:(n * 4 + tt + 1) * 128,
                                 hf * 260:(hf + 1) * 260],
                            vst[:, :])

            # ---- phase B: attention, two heads interleaved ----
            def emit_head_qs(h, vh, qs):
                po = (h % 2) * 64   # partition offset inside chunk
                mc = h // 2         # chunk index for this head
                cps = psC.tile([DK + 1, QS], F32, tag="cps", name=f"cps{h}_{qs}")
                nkb = 4 * qs + 4
                pts = [None] * nkb
                c0s = [None] * nkb

                def emit_st(kb):
                    col0 = max(0, kb * 128 - qs * QS)
                    sp = psA.tile([128, QS], F32, tag="ps", name=f"sp{h}_{qs}_{kb}")
                    nc.tensor.matmul(
                        sp[:, col0:],
                        _r(kt_s[po:po + 64, mc, kb * 128:(kb + 1) * 128]),
                        _r(qt_s[po:po + 64, mc,
                                qs * QS + col0:(qs + 1) * QS]),
                        start=True, stop=True)
                    pt = ptp.tile([128, QS], F32R, tag="pt", name=f"pt{h}_{qs}_{kb}")
                    nc.scalar.activation(
                        pt[:, col0:], sp[:, col0:],
                        func=mybir.ActivationFunctionType.Exp)
                    if col0 > 0 or kb == 4 * qs:
                        nc.vector.tensor_mul(
                            pt[:, col0:col0 + 128],
                            pt[:, col0:col0 + 128], msk_s[:, :])
                    pts[kb] = pt
                    c0s[kb] = col0

                def emit_av(kb):
                    col0 = c0s[kb]
                    nc.tensor.matmul(
                        cps[:, col0:],
                        _r(vh[:, kb, :]),
                        _r(pts[kb][:, col0:]),
                        start=(kb == 0), stop=(kb == nkb - 1))

                emit_st(0)
                emit_st(1)
                for kb in range(2, nkb):
                    emit_st(kb)
                    emit_av(kb - 2)
                emit_av(nkb - 2)
                emit_av(nkb - 1)

                rc = smallp.tile([128, QS], F32, tag="rc", name=f"rc{h}_{qs}")
                nc.vector.reciprocal(rc[64:65, :], cps[64:65, :])
                dn = dnbp.tile([1, QS], F32, tag="dn", name=f"dn{h}_{qs}")
                nc.sync.dma_start(dn[0:1, :], rc[64:65, :])
                bc = smallp.tile([64, QS], F32, tag="bc", name=f"bc{h}_{qs}")
                nc.sync.dma_start(bc[:, :],
                                  dn[0:1, :].partition_broadcast(64))
                co = workp.tile([64, QS], F32R, tag="co", name=f"co{h}_{qs}")
                nc.vector.tensor_mul(co[:, :], cps[0:64, :], bc[:, :])
                nc.sync.dma_start(
                    ctx_d[h * 64:(h + 1) * 64, qs * QS:(qs + 1) * QS],
                    co[:, :])

            for hp in (range(HD // 2) if "B" in phases else []):
                h0, h1 = 2 * hp, 2 * hp + 1
                vhs = []
                for h in (h0, h1):
                    vh = vhp.tile([128, NKB, DK + 1], F32R, tag="vh",
                                  name=f"vh{h}")
                    nc.sync.dma_start(
                        vh[:, :, :],
                        vg_d[:, h * 65:(h + 1) * 65].rearrange(
                            "(t p) a -> p t a", p=128))
                    vhs.append(vh)
                for qs in range(NQS):
                    emit_head_qs(h0, vhs[0], qs)
                    emit_head_qs(h1, vhs[1], qs)

            # ---- phase C: output projection ----
            for t in (range(NTT) if "C" in phases else []):
                cts = []
                for c in range(4):
                    ct = workp.tile([128, 128], F32R, tag="ct", bufs=8)
                    nc.sync.dma_start(
                        ct[:, :], ctx_d[c * 128:(c + 1) * 128,
                                        t * 128:(t + 1) * 128])
                    cts.append(ct)
                pss = [psA.tile([128, 512], F32, tag="ps", name=f"psC{i}") for i in range(2)]
                for c in range(4):
                    for n2 in range(2):
                        nc.tensor.matmul(
                            pss[n2][:, :], _r(cts[c][:, :]),
                            _r(wo_s[:, c, n2 * 512:(n2 + 1) * 512]),
                            start=(c == 0), stop=(c == 3))
                for n2 in range(2):
                    ot = outsp.tile([128, 512], F32, tag="ot")
                    nc.vector.tensor_copy(ot[:, :], pss[n2][:, :])
                    nc.sync.dma_start(
                        outp[t * 128:(t + 1) * 128,
                             n2 * 512:(n2 + 1) * 512], ot[:, :])

            if dbg:
                nc.sync.dma_start(qt_dbg[:, :], qt_s[:, :, :].bitcast(F32))
                nc.sync.dma_start(kt_dbg[:, :], kt_s[:, :, :].bitcast(F32))
                nc.sync.dma_start(vg_dbg[:, :], vg_d[:, :].bitcast(F32))
                nc.sync.dma_start(ctx_dbg[:, :], ctx_d[:, :].bitcast(F32))

    nc.compile()
    return nc


_NC = None
LAST_RESULTS = None


def kernel(**inputs):
    global _NC, LAST_RESULTS
    import os
    if _NC is None:
        _NC = _build_nc()

    f = lambda a: np.asarray(a, dtype=np.float32)
    q, k, v = f(inputs["q"]), f(inputs["k"]), f(inputs["v"])
    wq_w, wq_b = f(inputs["wq_w"]), f(inputs["wq_b"])
    wk_w, wk_b = f(inputs["wk_w"]), f(inputs["wk_b"])
    wv_w, wv_b = f(inputs["wv_w"]), f(inputs["wv_b"])
    wo_w, wo_b = f(inputs["wo_w"]), f(inputs["wo_b"])

    msk = np.ascontiguousarray(
        (np.arange(128)[None, :] >= np.arange(128)[:, None]).astype(np.float32))

    gmaps = []
    for g in range(2):
        sl = slice(g * GW, (g + 1) * GW)
        wqT = np.ascontiguousarray((wq_w[sl] * 0.125).T)
        wkT = np.ascontiguousarray(wk_w[sl].T)
        wvT = np.zeros((D, AUGW), np.float32)
        vbias = np.zeros((AUGW,), np.float32)
        for h in range(HD):
            wvT[:, h * 65:h * 65 + 64] = wv_w[g * GW + h * 64:
                                              g * GW + (h + 1) * 64].T
            vbias[h * 65:h * 65 + 64] = wv_b[g * GW + h * 64:
                                             g * GW + (h + 1) * 64]
            vbias[h * 65 + 64] = 1.0
        woT = np.ascontiguousarray(wo_w[:, sl].T)
        bqT = np.ascontiguousarray(
            (wq_b[sl] * 0.125).reshape(4, 128).T)
        bkT = np.ascontiguousarray(wk_b[sl].reshape(4, 128).T)
        gmaps.append(dict(wq=wqT, wk=wkT, wv=wvT, wo=woT, bq=bqT, bk=bkT,
                          vb=vbias, msk=msk))

    bmaps = []
    for b in range(B):
        bmaps.append(dict(
            xq=np.ascontiguousarray(q[b].T),
            xk=np.ascontiguousarray(k[b].T),
            xv=np.ascontiguousarray(v[b].T)))

    in_maps = [dict(**bmaps[c // 2], **gmaps[c % 2]) for c in range(8)]

    trace = bool(int(os.environ.get("KERNEL_TRACE", "0")))
    res = run_bass_kernel_spmd(_NC, in_maps, list(range(8)), trace=trace)
    LAST_RESULTS = res

    out = np.empty((B, L, D), np.float32)
    for b in range(B):
        out[b] = (res.results[2 * b]["outp"] + res.results[2 * b + 1]["outp"]
                  + wo_b[None, :])
    return out



# revision 11
# speedup vs baseline: 1.4841x; 1.4841x over previous
"""Causal MHA (B=4, L=2048, D=1024, H=16) on 8 NeuronCores.

Sharding: core c -> (batch b = c//2, head-group g = c%2). Data-parallel over
the 4 batches, tensor-parallel over heads (8 heads per core): wq/wk/wv
column-parallel, wo row-parallel. Each core returns a partial [L, D] output;
the host sums the two head-group partials per batch and adds wo_b.

Single fused streaming kernel, no DRAM round-trips:
  per 512-token slice n:
    A(n):  QT/KT = w @ x.T (f32r, psum-chunked), V_aug = x.T @ wv_aug (bf16)
           kept in SBUF ([128,4,L] kt, per-slice qt, [128,16,520] vaug).
    B(n):  per head h: S.T[keys,q] = KT_h.T @ QT_h (f32r), exp on ACT ->
           pt bf16, tri-mask on diag tile (DVE), flipped AV:
           avps[q, 4t, 65] += pt_block.T @ vaug (bf16, ones col = denom).
           Normalize with per-partition reciprocal (tensor_scalar_mul).
    T(n):  ctxn [q,512] -> ctxT [d,q] via DMA-transpose (xbar).
    C(n):  out[tok,1024] = sum_c ctxT[c].T @ wo[c] (bf16), interleaved
           into B(n+1) heads to hide latency.
  A(n+1) projection chunks are interleaved between B(n) heads so the PE
  stays busy while ACT works through exp, and input DMAs prefetch a full
  slice ahead.
"""

import numpy as np
import ml_dtypes

import concourse.bacc as bacc
import concourse.bass as bass
import concourse.mybir as mybir
import concourse.tile as tile
from concourse.bass_utils import run_bass_kernel_spmd

F32 = mybir.dt.float32
F32R = mybir.dt.float32r
BF16 = mybir.dt.bfloat16

B, L, D, H, DK = 4, 2048, 1024, 16, 64
HD = 8             # heads per core
GW = 512           # head-group width (8 heads * 64)
AUGW = HD * (DK + 1)   # 520: per head 64 dims + ones col (ones LAST per head)
NCH = D // 128     # 8 contraction chunks
NSL = 4            # token slices of 512
NTT = L // 128     # 16 token tiles


def _build_nc(dbg=False):
    nc = bacc.Bacc("TRN2", target_bir_lowering=False, debug=False, num_devices=8)

    xq = nc.dram_tensor("xq", [128, NCH, L], F32R, kind="ExternalInput").ap()
    xk = nc.dram_tensor("xk", [128, NCH, L], F32R, kind="ExternalInput").ap()
    xv = nc.dram_tensor("xv", [128, NCH, L], BF16, kind="ExternalInput").ap()
    wq = nc.dram_tensor("wq", [128, NCH, GW], F32R, kind="ExternalInput").ap()
    wk = nc.dram_tensor("wk", [128, NCH, GW], F32R, kind="ExternalInput").ap()
    wv = nc.dram_tensor("wv", [128, NCH, AUGW], BF16, kind="ExternalInput").ap()
    wo = nc.dram_tensor("wo", [128, 4, D], BF16, kind="ExternalInput").ap()
    bq = nc.dram_tensor("bq", [128, 4], F32, kind="ExternalInput").ap()
    bk = nc.dram_tensor("bk", [128, 4], F32, kind="ExternalInput").ap()
    vb = nc.dram_tensor("vb", [AUGW], F32, kind="ExternalInput").ap()
    msk = nc.dram_tensor("msk", [128, 128], BF16, kind="ExternalInput").ap()
    outp = nc.dram_tensor("outp", [L, D], F32, kind="ExternalOutput").ap()
    if dbg:
        qt_dbg = nc.dram_tensor("qt_dbg", [128, 4, GW], F32,
                                kind="ExternalOutput").ap()
        kt_dbg = nc.dram_tensor("kt_dbg", [128, 4, L], F32,
                                kind="ExternalOutput").ap()
        vg_dbg = nc.dram_tensor("vg_dbg", [128, NTT, AUGW], BF16,
                                kind="ExternalOutput").ap()
        cn_dbg = nc.dram_tensor("cn_dbg", [128, NTT, GW], BF16,
                                kind="ExternalOutput").ap()
        ct_dbg = nc.dram_tensor("ct_dbg", [128, NTT, 4, 128], BF16,
                                kind="ExternalOutput").ap()

    with tile.TileContext(nc) as tc:
        with (
            tc.tile_pool(name="persist", bufs=1) as persist,
            tc.tile_pool(name="qtp", bufs=(4 if dbg else 2)) as qtp,
            tc.tile_pool(name="xqk", bufs=(2 if dbg else 4)) as xqkp,
            tc.tile_pool(name="xvp", bufs=2) as xvp,
            tc.tile_pool(name="ptp", bufs=(4 if dbg else 6)) as ptp,
            tc.tile_pool(name="ctxn", bufs=(16 if dbg else 6)) as ctxnp,
            tc.tile_pool(name="ctxT", bufs=(16 if dbg else 4)) as ctxTp,
            tc.tile_pool(name="rcp", bufs=4) as rcp,
            tc.tile_pool(name="outs", bufs=2) as outsp,
            tc.tile_pool(name="psS", bufs=3, space="PSUM") as psS,
            tc.tile_pool(name="psAV", bufs=1, space="PSUM") as psAV,
            tc.tile_pool(name="psA", bufs=2, space="PSUM") as psA,
            tc.tile_pool(name="psC", bufs=2, space="PSUM") as psC,
        ):
            # ---- persistent SBUF ----
            wq_s = persist.tile([128, NCH, GW], F32R, tag="wq")
            wk_s = persist.tile([128, NCH, GW], F32R, tag="wk")
            wv_s = persist.tile([128, NCH, AUGW], BF16, tag="wv")
            wo_s = persist.tile([128, 4, D], BF16, tag="wo")
            kt_s = persist.tile([128, 4, L], F32R, tag="kt")
            vaug_s = persist.tile([128, NTT, AUGW], BF16, tag="vaug")
            bq_s = persist.tile([128, 4], F32, tag="bq")
            bk_s = persist.tile([128, 4], F32, tag="bk")
            vb_s = persist.tile([128, AUGW], F32, tag="vb")
            msk_s = persist.tile([128, 128], BF16, tag="msk")

            # weight/const loads; order = DMA engine order (startup latency)
            nc.sync.dma_start(bq_s[:, :], bq[:, :])
            nc.sync.dma_start(bk_s[:, :], bk[:, :])
            nc.sync.dma_start(wq_s[:, :, :], wq[:, :, :])
            nc.sync.dma_start(wk_s[:, :, :], wk[:, :, :])

            xq_tiles = {}   # (slice, half) -> tile
            xk_tiles = {}
            xv_tiles = {}   # slice -> tile [128, NCH, 512] bf16

            def issue_xin(n):
                c0, c1 = n * 512, (n + 1) * 512
                for hf in range(2):
                    t = xqkp.tile([128, NCH, 256], F32R, tag="xqk",
                                  name=f"xq{n}_{hf}")
                    nc.sync.dma_start(
                        t[:, :, :], xq[:, :, c0 + hf * 256:c0 + hf * 256 + 256])
                    xq_tiles[(n, hf)] = t
                    t = xqkp.tile([128, NCH, 256], F32R, tag="xqk",
                                  name=f"xk{n}_{hf}")
                    nc.sync.dma_start(
                        t[:, :, :], xk[:, :, c0 + hf * 256:c0 + hf * 256 + 256])
                    xk_tiles[(n, hf)] = t
                t = xvp.tile([128, NCH, 512], BF16, tag="xv", name=f"xv{n}")
                nc.sync.dma_start(t[:, :, :], xv[:, :, c0:c1])
                xv_tiles[n] = t

            issue_xin(0)
            nc.sync.dma_start(wv_s[:, :, :], wv[:, :, :])
            nc.sync.dma_start(msk_s[:, :], msk[:, :])
            vb_bcast = bass.AP(tensor=vb.tensor, offset=vb.offset,
                               ap=[[0, 128], [1, AUGW]])
            nc.gpsimd.dma_start(vb_s[:, :], vb_bcast)
            nc.sync.dma_start(wo_s[:, :, :], wo[:, :, :])

            qt_tiles = {}

            # ---- phase A unit generator: projections for slice n ----
            def a_units(n):
                qt_t = qtp.tile([128, 4, GW], F32R, tag="qt", name=f"qt{n}")
                qt_tiles[n] = qt_t

                def qk_unit(hf, g, x_t, w_s, b_s, is_q):
                    def emit():
                        ps = psA.tile([128, 2, 256], F32, tag="pa",
                                      name=f"pa{n}_{hf}_{g}")
                        for c in range(NCH):
                            for mi in range(2):
                                # start=True zeroes the whole psum bank, so
                                # only the first write into the tile sets it
                                nc.tensor.matmul(
                                    ps[:, mi, :],
                                    w_s[:, c, (2 * g + mi) * 128:
                                        (2 * g + mi + 1) * 128],
                                    x_t[:, c, :],
                                    start=(c == 0 and mi == 0),
                                    stop=(c == NCH - 1),
                                    skip_group_check=True)
                        for mi in range(2):
                            m = 2 * g + mi
                            if is_q:
                                nc.vector.tensor_scalar_add(
                                    qt_t[:, m, hf * 256:hf * 256 + 256],
                                    ps[:, mi, :], b_s[:, m:m + 1])
                            else:
                                nc.vector.tensor_scalar_add(
                                    kt_s[:, m, n * 512 + hf * 256:
                                         n * 512 + hf * 256 + 256],
                                    ps[:, mi, :], b_s[:, m:m + 1])
                    return emit

                def v_unit(tt, vhf):
                    def emit():
                        ps = psA.tile([128, 260], F32, tag="pa",
                                      name=f"pv{n}_{tt}_{vhf}")
                        xv_t = xv_tiles[n]
                        for c in range(NCH):
                            nc.tensor.matmul(
                                ps[:, :],
                                xv_t[:, c, tt * 128:(tt + 1) * 128],
                                wv_s[:, c, vhf * 260:(vhf + 1) * 260],
                                start=(c == 0), stop=(c == NCH - 1))
                        nc.vector.tensor_add(
                            vaug_s[:, n * 4 + tt, vhf * 260:(vhf + 1) * 260],
                            ps[:, :], vb_s[:, vhf * 260:(vhf + 1) * 260])
                    return emit

                units = []
                for hf in range(2):
                    for g in range(2):
                        units.append(qk_unit(hf, g, xq_tiles[(n, hf)],
                                             wq_s, bq_s, True))
                    for g in range(2):
                        units.append(qk_unit(hf, g, xk_tiles[(n, hf)],
                                             wk_s, bk_s, False))
                    for tt in (2 * hf, 2 * hf + 1):
                        for vhf in range(2):
                            units.append(v_unit(tt, vhf))
                return units

            ctxn_tiles = {}   # (n, j) -> tile
            ctxT_tiles = {}   # (n, j) -> tile

            # ---- phase B: one head of slice n ----
            def b_head(n, h):
                po = (h % 2) * 64
                mc = h // 2
                qt_t = qt_tiles[n]
                nkb = 4 * n + 4
                avps = psAV.tile([128, 4, DK + 1], F32, tag="av",
                                 name=f"av{n}_{h}")
                pts = [None] * nkb
                c0s = [None] * nkb

                def emit_s(kb):
                    jj = kb - 4 * n
                    col0e = max(0, jj * 128)      # valid/exp start
                    col0s = min(col0e, 256)       # S start (keep N >= 256)
                    sp = psS.tile([128, 512], F32, tag="sp",
                                  name=f"sp{n}_{h}_{kb}")
                    nc.tensor.matmul(
                        sp[:, col0s:],
                        kt_s[po:po + 64, mc, kb * 128:(kb + 1) * 128],
                        qt_t[po:po + 64, mc, col0s:],
                        start=True, stop=True)
                    pt = ptp.tile([128, 512], BF16, tag="pt",
                                  name=f"pt{n}_{h}_{kb}")
                    nc.scalar.activation(
                        pt[:, col0e:], sp[:, col0e:],
                        func=mybir.ActivationFunctionType.Exp)
                    if jj >= 0:
                        nc.vector.tensor_mul(
                            pt[:, col0e:col0e + 128],
                            pt[:, col0e:col0e + 128], msk_s[:, :])
                    pts[kb] = pt
                    c0s[kb] = col0e

                def emit_av(kb):
                    j0 = max(0, kb - 4 * n)
                    for j in range(j0, 4):
                        # whole-bank zero on start: only (kb=0, j=0) sets it
                        nc.tensor.matmul(
                            avps[:, j, :],
                            pts[kb][:, j * 128:(j + 1) * 128],
                            vaug_s[:, kb, h * 65:(h + 1) * 65],
                            start=(kb == 0 and j == 0),
                            stop=(kb == 4 * n + j),
                            skip_group_check=True)

                emit_s(0)
                if nkb > 1:
                    emit_s(1)
                for kb in range(2, nkb):
                    emit_s(kb)
                    emit_av(kb - 2)
                emit_av(nkb - 2)
                emit_av(nkb - 1)

                rc = rcp.tile([128, 4], F32, tag="rc", name=f"rc{n}_{h}")
                nc.vector.reciprocal(rc[:, :], avps[:, :, 64])
                for j in range(4):
                    nc.vector.tensor_scalar_mul(
                        ctxn_tiles[(n, j)][:, h * 64:(h + 1) * 64],
                        avps[:, j, 0:64], rc[:, j:j + 1])

            # ---- phase C unit: token tile t, output half n2 ----
            out_tiles = {}

            def c_unit(n, j, n2):
                t = 4 * n + j

                def emit():
                    if n2 == 0:
                        out_tiles[t] = outsp.tile([128, D], F32, tag="outs",
                                                  name=f"out{t}")
                    cps = psC.tile([128, 512], F32, tag="cps",
                                   name=f"cps{t}_{n2}")
                    ctxT_t = ctxT_tiles[(n, j)]
                    for c in range(4):
                        nc.tensor.matmul(
                            cps[:, :], ctxT_t[:, c, :],
                            wo_s[:, c, n2 * 512:(n2 + 1) * 512],
                            start=(c == 0), stop=(c == 3))
                    nc.vector.tensor_copy(
                        out_tiles[t][:, n2 * 512:(n2 + 1) * 512], cps[:, :])
                    if n2 == 1:
                        nc.sync.dma_start(
                            outp[t * 128:(t + 1) * 128, :], out_tiles[t][:, :])
                return emit

            # ---- main schedule ----
            for u in a_units(0):
                u()

            pending_c = []
            for n in range(NSL):
                if n < NSL - 1:
                    issue_xin(n + 1)
                for j in range(4):
                    ctxn_tiles[(n, j)] = ctxnp.tile(
                        [128, GW], BF16, tag="ctxn", name=f"ctxn{n}_{j}")
                au = a_units(n + 1) if n < NSL - 1 else []
                ai = 0
                for h in range(HD):
                    b_head(n, h)
                    for _ in range(2):
                        if ai < len(au):
                            au[ai]()
                            ai += 1
                    if pending_c:
                        pending_c.pop(0)()
                while ai < len(au):
                    au[ai]()
                    ai += 1
                while pending_c:
                    pending_c.pop(0)()
                for j in range(4):
                    ct = ctxTp.tile([128, 4, 128], BF16, tag="ctxT",
                                    name=f"ctxT{n}_{j}")
                    nc.sync.dma_start_transpose(ct, ctxn_tiles[(n, j)][:, :])
                    ctxT_tiles[(n, j)] = ct
                for j in range(4):
                    for n2 in range(2):
                        pending_c.append(c_unit(n, j, n2))
            while pending_c:
                pending_c.pop(0)()

            if dbg:
                nc.sync.dma_start(qt_dbg[:, :, :],
                                  qt_tiles[0][:, :, :].bitcast(F32))
                nc.sync.dma_start(kt_dbg[:, :, :], kt_s[:, :, :].bitcast(F32))
                nc.sync.dma_start(vg_dbg[:, :, :], vaug_s[:, :, :])
                for n in range(NSL):
                    for j in range(4):
                        nc.sync.dma_start(cn_dbg[:, 4 * n + j, :],
                                          ctxn_tiles[(n, j)][:, :])
                        nc.sync.dma_start(ct_dbg[:, 4 * n + j, :, :],
                                          ctxT_tiles[(n, j)][:, :, :])

    nc.compile()
    return nc


_NC = None
LAST_RESULTS = None


def kernel(**inputs):
    global _NC, LAST_RESULTS
    import os
    if _NC is None:
        _NC = _build_nc()

    f = lambda a: np.asarray(a, dtype=np.float32)
    q, k, v = f(inputs["q"]), f(inputs["k"]), f(inputs["v"])
    wq_w, wq_b = f(inputs["wq_w"]), f(inputs["wq_b"])
    wk_w, wk_b = f(inputs["wk_w"]), f(inputs["wk_b"])
    wv_w, wv_b = f(inputs["wv_w"]), f(inputs["wv_b"])
    wo_w, wo_b = f(inputs["wo_w"]), f(inputs["wo_b"])

    bf = ml_dtypes.bfloat16

    def chunk_rows(a, inner):
        # [1024, X] -> [128, 8, X] with row r = c*128+p -> [p, c, :]
        return np.ascontiguousarray(
            a.reshape(NCH, 128, inner).transpose(1, 0, 2))

    msk = np.ascontiguousarray(
        (np.arange(128)[None, :] >= np.arange(128)[:, None])).astype(bf)

    gmaps = []
    for g in range(2):
        sl = slice(g * GW, (g + 1) * GW)
        wqT = chunk_rows((wq_w[sl] * 0.125).T, GW)
        wkT = chunk_rows(wk_w[sl].T, GW)
        wvT = np.zeros((D, AUGW), np.float32)
        vbias = np.zeros((AUGW,), np.float32)
        for h in range(HD):
            wvT[:, h * 65:h * 65 + 64] = wv_w[g * GW + h * 64:
                                              g * GW + (h + 1) * 64].T
            vbias[h * 65:h * 65 + 64] = wv_b[g * GW + h * 64:
                                             g * GW + (h + 1) * 64]
            vbias[h * 65 + 64] = 1.0
        woT = np.ascontiguousarray(
            wo_w[:, sl].T.reshape(4, 128, D).transpose(1, 0, 2)).astype(bf)
        bqT = np.ascontiguousarray((wq_b[sl] * 0.125).reshape(4, 128).T)
        bkT = np.ascontiguousarray(wk_b[sl].reshape(4, 128).T)
        gmaps.append(dict(wq=wqT, wk=wkT, wv=chunk_rows(wvT, AUGW).astype(bf),
                          wo=woT, bq=bqT, bk=bkT, vb=vbias, msk=msk))

    bmaps = []
    for b in range(B):
        bmaps.append(dict(
            xq=chunk_rows(np.ascontiguousarray(q[b].T), L),
            xk=chunk_rows(np.ascontiguousarray(k[b].T), L),
            xv=chunk_rows(np.ascontiguousarray(v[b].T), L).astype(bf)))

    in_maps = [dict(**bmaps[c // 2], **gmaps[c % 2]) for c in range(8)]

    trace = bool(int(os.environ.get("KERNEL_TRACE", "0")))
    res = run_bass_kernel_spmd(_NC, in_maps, list(range(8)), trace=trace)
    LAST_RESULTS = res

    out = np.empty((B, L, D), np.float32)
    for b in range(B):
        out[b] = (res.results[2 * b]["outp"] + res.results[2 * b + 1]["outp"]
                  + wo_b[None, :])
    return out
